# revision 30
# baseline (speedup 1.0000x reference)
"""Distributed Bass kernel for nn_Attention (B=2, T=2048, D=1024, H=16) on 8 TRN2 cores.

Sharding: core c -> (batch b = c//4, head-group g = c%4, heads 4g..4g+3).
QKV tensor-parallel over heads; out-proj COLUMN-parallel: AllGather the
(small) attention outputs per token chunk, then each core computes its own
256-col slice of the output locally -- no ReduceScatter, no final collective,
and AG moves 1/4 the bytes RS did.

v4:
  - rowsum rides the av matmul as 64 ones-columns (m: 65->128; matmul time
    is free-dim-bound so this is free); po is evicted to SBUF in one copy
    (frees the PSUM accumulator for the next head-pair ~2.5us earlier) and
    normalize = fast ~18-bit DVE reciprocal + multiply, all off-PE.
  - sc[j+1] enqueued before av[j]: the in-order PE queue never stalls on exp.
  - scalar engine runs only Exp in attention (1 act table load total).
  - last 512 tokens processed as two 256-wide blocks so the final AG +
    out-proj overlap the preceding attention (tail ~45us -> ~18us).
  - host pre-tiles xT/whT/WT to [128, n, ...] so input DMAs are contiguous
    per partition; a dummy warm-up AllGather absorbs the ncfw cold start.
"""

import functools
import numpy as np
from contextlib import ExitStack

B, T, D, H, HD = 2, 2048, 1024, 16, 64
EPS = 1e-4
NCORES, GROUP = 8, 4
HL = H // GROUP          # heads per core = 4
DL = HL * HD             # local feature cols = 256
NTT = T // 128           # 16 token tiles
NDT = D // 128           # 8 d tiles
WCOLS = 3 * DL           # 768 qkv output cols per core

# attention q-blocks (tok0, width); the two row-tiled score matmuls write
# PSUM offsets {0, BANKW} so each lands in its own bank even at width 256.
BLOCKS = [(1536, 256), (0, 512), (512, 512), (1024, 512), (1792, 256)]
BANKW = 512
# QKV token-tile processing order: the 1536-2047 quarter first so its q/k
# transposes are ready when attention (which starts with those queries)
# begins; the 1024-1535 quarter last (only needed late).
TT_ORDER = [12, 13, 14, 15] + list(range(8)) + [8, 9, 10, 11]
# attention key-tile visit order: defer the 1024-1535 keys (their transposes
# land right at the QKV->attention boundary)
J_ORDER = list(range(8)) + [12, 13, 14, 15] + [8, 9, 10, 11]
# scheduler not-before hints (in ms of simulated time) for each chunk's
# out-proj burst: the cost model underestimates the (cold) AllGather latency,
# so without these the scheduler places the bursts too early and the in-order
# PE queue stalls on the aoTf load.
WAITS = [0.132, 0.158, 0.188, 0.228]


def _build_bass():
    import concourse.bass as bass
    import concourse.tile as tile
    from concourse import bacc, mybir

    f32 = mybir.dt.float32
    bf16 = mybir.dt.bfloat16
    AX = mybir.AxisListType
    OP = mybir.AluOpType
    AF = mybir.ActivationFunctionType

    nc = bacc.Bacc("TRN2", target_bir_lowering=False, debug=False, num_devices=NCORES)

    xT_ext = nc.dram_tensor("xT", [128, NDT, T], bf16, kind="ExternalInput").ap()
    whT_ext = nc.dram_tensor("whT", [128, NDT, WCOLS], bf16, kind="ExternalInput").ap()
    WT_ext = nc.dram_tensor("WT", [128, NDT, DL], bf16, kind="ExternalInput").ap()
    out_ext = nc.dram_tensor("out", [DL, T], bf16, kind="ExternalOutput").ap()

    with tile.TileContext(nc) as tc, ExitStack() as ctx:
        # ---------------- persistent pools ----------------
        pers = ctx.enter_context(tc.tile_pool(name="pers", bufs=1))
        dram = ctx.enter_context(tc.tile_pool(name="dram", bufs=1, space="DRAM"))

        warm_sb = pers.tile([128, 640], bf16)
        xT_sb = pers.tile([128, NDT, T], bf16)
        whT_sb = pers.tile([128, NDT, WCOLS], bf16)
        WT_sb = pers.tile([128, NDT, DL], bf16)
        qT_sb = [[pers.tile([128, 512], bf16, name=f"qT{rb}_{tq}")
                  for tq in range(4)] for rb in range(2)]
        kT_sb = [[pers.tile([128, 512], bf16, name=f"kT{rb}_{tq}")
                  for tq in range(4)] for rb in range(2)]
        # v + 64 ones-columns per (token-tile, head): av lhsT = [v | 1]
        v_sb = pers.tile([128, NTT, HL, 128], bf16)
        aoT_sb = [pers.tile([128, T], bf16, name=f"aoT{rb}") for rb in range(2)]

        qknat = dram.tile([T, 2 * DL], bf16)

        ag_in = [dram.tile([DL, w], bf16, name=f"ag_in{k}")
                 for k, (_, w) in enumerate(BLOCKS)]
        ag_out = [dram.tile([D, w], bf16, name=f"ag_out{k}")
                  for k, (_, w) in enumerate(BLOCKS)]

        # warmup source + the ones columns of v (one strided memset each)
        nc.vector.memset(warm_sb[:], 1.0)
        nc.vector.memset(v_sb[:, :, :, 64:128], 1.0)

        # ---------------- input DMAs (contiguous, split across queues) ----
        # balance the QKV-critical bytes (whT full + x first chunk) across
        # the three DMA-capable queues; all transfers are contiguous per
        # partition thanks to the host pre-tiling
        nc.sync.dma_start(whT_sb[:, 0:4, :], whT_ext[:, 0:4, :])
        nc.scalar.dma_start(whT_sb[:, 4:8, :], whT_ext[:, 4:8, :])
        nc.scalar.dma_start(WT_sb[:], WT_ext)
        for xc in (3, 0, 1, 2):  # match TT_ORDER consumption
            nc.gpsimd.dma_start(
                xT_sb[:, :, 512 * xc : 512 * (xc + 1)],
                xT_ext[:, :, 512 * xc : 512 * (xc + 1)])

        # ---------------- QKV + attention (one pool scope: no pool-close
        # barrier between the phases) ----------------
        with tc.tile_pool(name="scps", bufs=2, space="PSUM") as scps, \
             tc.tile_pool(name="pops", bufs=1, space="PSUM") as pops, \
             tc.tile_pool(name="ypps", bufs=1, space="PSUM") as ypps, \
             tc.tile_pool(name="qsb", bufs=3) as qsb, \
             tc.tile_pool(name="exsb", bufs=4) as exsb, \
             tc.tile_pool(name="posb", bufs=2) as posb, \
             tc.tile_pool(name="aosb", bufs=2) as aosb, \
             tc.tile_pool(name="ysb", bufs=2) as ysb:
            # PE warm-up: dependency-light matmuls while x/whT stream in
            # (uses the po ring so no extra PSUM pool is needed)
            for wu in range(12):
                wt = pops.tile([128, 2 * BANKW], f32, name="po")
                nc.tensor.matmul(wt[:, 0:512], warm_sb[:, 0:128],
                                 warm_sb[:, 128:640], start=True, stop=True)

            done_tq = [0, 0, 0, 0]
            for tt in TT_ORDER:
                ps = scps.tile([128, 2 * BANKW], f32, name="sc")
                for dt_ in range(NDT):
                    lhsT = xT_sb[:, dt_, 128 * tt : 128 * (tt + 1)]
                    nc.tensor.matmul(ps[:, 0:512], lhsT, whT_sb[:, dt_, 0:512],
                                     start=(dt_ == 0), stop=(dt_ == NDT - 1))
                    nc.tensor.matmul(ps[:, 512:768], lhsT, whT_sb[:, dt_, 512:768],
                                     start=(dt_ == 0), stop=(dt_ == NDT - 1))
                # evict q+k raw and v quickly so ps recycles (ACT, 2 instrs)
                qk_sb = qsb.tile([128, 2 * DL], bf16, name="qk_sb")
                nc.scalar.activation(qk_sb[:], ps[:, 0 : 2 * DL], AF.Copy)
                nc.vector.tensor_copy(
                    v_sb[:, tt, :, 0:64],
                    ps[:, 2 * DL : 3 * DL].rearrange("p (h c) -> p h c", c=HD))
                # per-head norms of q and k off the SBUF copy
                sq = qsb.tile([128, 2 * DL], bf16, name="sq")
                nc.vector.tensor_tensor(sq[:], qk_sb[:], qk_sb[:], op=OP.mult)
                ns = qsb.tile([128, 2 * HL], f32, name="ns")
                nc.vector.reduce_sum(
                    ns[:], sq[:].rearrange("p (h c) -> p h c", c=HD), axis=AX.X)
                # sqrt(sumsq/64) = |q|/8 for q; sqrt(sumsq) = |k| for k
                nc.scalar.activation(ns[:, 0:HL], ns[:, 0:HL], AF.Sqrt,
                                     scale=1.0 / 64.0)
                nc.scalar.activation(ns[:, HL : 2 * HL], ns[:, HL : 2 * HL],
                                     AF.Sqrt)
                inv = qsb.tile([128, 2 * HL], f32, name="inv")
                nc.vector.reciprocal(inv[:], ns[:])
                # qst = q * 8/|q| ; kst = k / |k|  (scale folded per head)
                qkst = qsb.tile([128, 2 * DL], bf16, name="qkst")
                nc.vector.tensor_tensor(
                    qkst[:].rearrange("p (h c) -> p h c", c=HD),
                    qk_sb[:].rearrange("p (h c) -> p h c", c=HD),
                    inv[:].rearrange("p (h o) -> p h o", o=1).broadcast_to(
                        (128, 2 * HL, HD)),
                    op=OP.mult)
                nc.sync.dma_start(qknat[128 * tt : 128 * (tt + 1), :], qkst[:])
                done_tq[tt // 4] += 1
                if done_tq[tt // 4] == 4:
                    tq = tt // 4
                    tsl = slice(512 * tq, 512 * (tq + 1))
                    for rb in range(2):
                        nc.sync.dma_start_transpose(
                            qT_sb[rb][tq][:],
                            qknat[tsl, 128 * rb : 128 * (rb + 1)])
                        nc.sync.dma_start_transpose(
                            kT_sb[rb][tq][:],
                            qknat[tsl, 256 + 128 * rb : 256 + 128 * (rb + 1)])

            # ---------- attention + overlapped AG/out-proj ----------
            # av matmuls trail the sc/exp stream by TWO iterations so the
            # in-order PE queue dispatches back-to-back (exp latency hidden).
            # Out-proj runs as one solid 16-matmul burst per chunk.

            def outproj_solid(aoTf, ci):
                """one solid PE run: 16 accumulating matmuls into a borrowed
                scps tile (mh halves in separate banks), evict, DMA out."""
                tok0c, wc = BLOCKS[ci]
                yp = ypps.tile([128, 2 * BANKW], f32, name="yp", tag="yp")
                for mh in range(2):
                    for dt_ in range(NDT):
                        nc.tensor.matmul(
                            yp[:, BANKW * mh : BANKW * mh + wc],
                            WT_sb[:, dt_, 128 * mh : 128 * (mh + 1)],
                            aoTf[:, dt_, 0:wc],
                            start=(dt_ == 0), stop=(dt_ == NDT - 1))
                yst = ysb.tile([128, 2 * BANKW], bf16, name="yst")
                nc.vector.tensor_copy(
                    yst[:, 0 : 2 * wc].rearrange("p (h w) -> p h w", h=2),
                    yp[:].rearrange("p (h w) -> p h w", h=2)[:, :, 0:wc])
                for mh in range(2):
                    nc.sync.dma_start(
                        out_ext[128 * mh : 128 * (mh + 1), tok0c : tok0c + wc],
                        yst[:, wc * mh : wc * (mh + 1)])

            git = 0  # global attention iteration counter
            for bi, (tok0, width) in enumerate(BLOCKS):
                for rb in range(2):
                    po = pops.tile([128, 2 * BANKW], f32, name="po")
                    pend = []  # (ex, j) issued ahead of their av matmuls
                    for idx in range(NTT + 2):
                        if idx < NTT:
                            j = J_ORDER[idx]
                            sc = scps.tile([128, 2 * BANKW], f32, name="sc")
                            jq, jr = j // 4, j % 4
                            qq, qr = tok0 // 512, tok0 % 512
                            for hh in range(2):
                                nc.tensor.matmul(
                                    sc[:, BANKW * hh : BANKW * hh + width],
                                    kT_sb[rb][jq][64 * hh : 64 * (hh + 1),
                                                  128 * jr : 128 * (jr + 1)],
                                    qT_sb[rb][qq][64 * hh : 64 * (hh + 1),
                                                  qr : qr + width],
                                    start=True, stop=True)
                            ex = exsb.tile([128, 2 * BANKW], bf16, name="ex")
                            nc.scalar.activation(
                                ex[:, 0 : 2 * width].rearrange(
                                    "p (h w) -> p h w", h=2),
                                sc[:].rearrange("p (h w) -> p h w",
                                                h=2)[:, :, 0:width],
                                AF.Exp)
                            pend.append(ex)
                        if idx >= 2:
                            jj = J_ORDER[idx - 2]
                            exd = pend.pop(0)
                            for hh in range(2):
                                nc.tensor.matmul(
                                    po[0:128, BANKW * hh : BANKW * hh + width],
                                    v_sb[:, jj, 2 * rb + hh, 0:128],
                                    exd[:, width * hh : width * (hh + 1)],
                                    start=(idx == 2), stop=(idx == NTT + 1))
                            git += 1
                    # evict po (frees the accumulator banks for the next
                    # head-pair), then normalize off SBUF: ao = av/rowsum
                    poc = posb.tile([64, 2 * BANKW], f32, name="poc")
                    nc.vector.tensor_copy(
                        poc[:, 0 : 2 * width].rearrange("p (h w) -> p h w", h=2),
                        po[0:64].rearrange("p (h w) -> p h w", h=2)[:, :, 0:width])
                    rsum = posb.tile([64, 2 * BANKW], f32, name="rsum")
                    nc.vector.tensor_copy(
                        rsum[:, 0 : 2 * width].rearrange("p (h w) -> p h w", h=2),
                        po[64:128].rearrange("p (h w) -> p h w",
                                             h=2)[:, :, 0:width])
                    rinv = posb.tile([64, 2 * BANKW], f32, name="rinv")
                    nc.vector.reciprocal_approx_fast(
                        rinv[:, 0 : 2 * width], rsum[:, 0 : 2 * width])
                    for hh in range(2):
                        nc.vector.tensor_tensor(
                            aoT_sb[rb][64 * hh : 64 * (hh + 1),
                                       tok0 : tok0 + width],
                            poc[:, width * hh : width * (hh + 1)],
                            rinv[:, width * hh : width * (hh + 1)],
                            op=OP.mult)
                # both rb done: ship ao chunk and AllGather it (gpsimd queue
                # orders the DMAs before the collective trigger)
                for rb in range(2):
                    nc.gpsimd.dma_start(
                        ag_in[bi][128 * rb : 128 * (rb + 1), :],
                        aoT_sb[rb][:, tok0 : tok0 + width])
                nc.gpsimd.collective_compute(
                    "AllGather", mybir.AluOpType.bypass,
                    replica_groups=[[0, 1, 2, 3], [4, 5, 6, 7]],
                    ins=[ag_in[bi].opt()], outs=[ag_out[bi].opt()])
                # load the gathered full-feature aoT (sync queue; waits on AG)
                aoTf = aosb.tile([128, NDT, 512], bf16, name="aoTf")
                nc.sync.dma_start(
                    aoTf[:, :, 0:width],
                    ag_out[bi].rearrange("(n p) t -> p n t", p=128))
                if bi < len(WAITS):
                    with tc.tile_wait_until(WAITS[bi]):
                        outproj_solid(aoTf, bi)
                else:
                    outproj_solid(aoTf, bi)

    nc.compile()
    return nc


@functools.lru_cache(maxsize=1)
def _get_nc():
    return _build_bass()


def _mp_normalize_rows(w):
    n = np.linalg.norm(w, axis=-1, keepdims=True)
    n = EPS + n * (1.0 / np.sqrt(w.shape[-1]))
    return (w / n) * (1.0 / np.sqrt(w.shape[-1]))


def _tile_p(a):
    """[128*n, c] row-major -> [128, n, c] with row = n*128 + p."""
    n = a.shape[0] // 128
    return np.ascontiguousarray(a.reshape(n, 128, *a.shape[1:]).swapaxes(0, 1))


def make_in_maps(x, w_qkv, w_out):
    import ml_dtypes

    x = np.asarray(x, dtype=np.float32)
    w_qkv = np.asarray(w_qkv, dtype=np.float32)
    w_out = np.asarray(w_out, dtype=np.float32)

    wq_hat = _mp_normalize_rows(w_qkv)           # (3D, D) row-normalized/32
    wo_hat = _mp_normalize_rows(w_out)           # (D, D)
    woT = np.ascontiguousarray(wo_hat.T)         # (D_in, D_out)

    in_maps = []
    for c in range(NCORES):
        b, g = c // GROUP, c % GROUP
        rows = np.concatenate([
            np.arange(DL * g, DL * (g + 1)),
            D + np.arange(DL * g, DL * (g + 1)),
            2 * D + np.arange(DL * g, DL * (g + 1)),
        ])
        whT = np.ascontiguousarray(wq_hat[rows].T)   # (D, 768)
        in_maps.append({
            "xT": _tile_p(np.ascontiguousarray(x[b].T)).astype(ml_dtypes.bfloat16),
            "whT": _tile_p(whT).astype(ml_dtypes.bfloat16),
            "WT": _tile_p(np.ascontiguousarray(
                woT[:, DL * g : DL * (g + 1)])).astype(ml_dtypes.bfloat16),
        })
    return in_maps


def kernel(x: np.ndarray, w_qkv: np.ndarray, w_out: np.ndarray) -> np.ndarray:
    from concourse.bass_utils import run_bass_kernel_spmd

    in_maps = make_in_maps(x, w_qkv, w_out)
    nc = _get_nc()
    res = run_bass_kernel_spmd(nc, in_maps, core_ids=list(range(NCORES)))

    out = np.empty((B, T, D), dtype=np.float32)
    for c in range(NCORES):
        b, g = c // GROUP, c % GROUP
        out[b][:, DL * g : DL * (g + 1)] = res.results[c]["out"].astype(np.float32).T
    return out


# revision 31
# speedup vs baseline: 1.0092x; 1.0092x over previous
"""Distributed Bass kernel for nn_Attention (B=2, T=2048, D=1024, H=16) on 8 TRN2 cores.

Sharding: core c -> (batch b = c//4, head-group g = c%4, heads 4g..4g+3).
QKV tensor-parallel over heads; out-proj COLUMN-parallel: per token chunk,
AllGather the (small, 128-256KB) normalized attention outputs across the
4-core group, then each core computes its own 256-col slice of the final
output locally -- no ReduceScatter, no collective after the last out-proj,
and ~4x less collective traffic than the row-parallel+RS formulation.

Key mechanics (see git history of this session for the journey):
  - rowsum rides the av matmul as 64 ones-columns in the lhsT (m: 65->128;
    matmul time is free-dim-bound so the row sums are free), so softmax
    normalize is two aligned PSUM->SBUF copies + a fast ~18-bit DVE
    reciprocal + multiply, entirely off the PE/ACT critical engines.
  - av matmuls trail the sc/exp stream by TWO iterations so the in-order PE
    queue never head-of-line blocks on the exp (ACT) latency.
  - scalar engine runs only Exp in attention (exactly one act-table load at
    the phase transition); q/k eviction+norms use ACT-copy/sqrt + DVE.
  - one PSUM pool scope across QKV and attention (same tile names): no
    all-engine pool-close barrier between the phases.
  - block order [1536x256, 0..1536 as 3x512, 1792x256]: the expensive cold
    first AllGather is triggered ~20us earlier on a cheap block and the
    final chunk's AllGather is the small one; QKV processes token tiles in
    TT_ORDER (last quarter first) so the q/k transposes needed by the first
    attention block are ready at the transition, and the attention visits
    key tiles in J_ORDER so the late-transposed quarter is needed last.
  - out-proj runs as one solid 16-matmul burst per chunk, placed via
    tc.tile_wait_until scheduler hints (the cost model underestimates the
    0.4-11us trigger delay + 9-36us execution of the AllGathers here, so
    dependency-driven placement would stall the in-order PE queue).
  - host pre-tiles xT/whT/WT to [128, n, ...] so all input DMAs are
    contiguous per partition; xT chunks are DMAed in TT_ORDER consumption
    order; whT is split across two DMA queues.
"""

import functools
import numpy as np
from contextlib import ExitStack

B, T, D, H, HD = 2, 2048, 1024, 16, 64
EPS = 1e-4
NCORES, GROUP = 8, 4
HL = H // GROUP          # heads per core = 4
DL = HL * HD             # local feature cols = 256
NTT = T // 128           # 16 token tiles
NDT = D // 128           # 8 d tiles
WCOLS = 3 * DL           # 768 qkv output cols per core

# attention q-blocks (tok0, width); the two row-tiled score matmuls write
# PSUM offsets {0, BANKW} so each lands in its own bank even at width 256.
BLOCKS = [(1536, 256), (0, 512), (512, 512), (1024, 512), (1792, 256)]
BANKW = 512
# QKV token-tile processing order: the 1536-2047 quarter first so its q/k
# transposes are ready when attention (which starts with those queries)
# begins; the 1024-1535 quarter last (only needed late).
TT_ORDER = [12, 13, 14, 15] + list(range(8)) + [8, 9, 10, 11]
# attention key-tile visit order: defer the 1024-1535 keys (their transposes
# land right at the QKV->attention boundary)
J_ORDER = list(range(8)) + [12, 13, 14, 15] + [8, 9, 10, 11]
# scheduler not-before hints (in ms of simulated time) for each chunk's
# out-proj burst: the cost model underestimates the (cold) AllGather latency,
# so without these the scheduler places the bursts too early and the in-order
# PE queue stalls on the aoTf load.
WAITS = [0.132, 0.158, 0.188, 0.228]


def _build_bass():
    import concourse.bass as bass
    import concourse.tile as tile
    from concourse import bacc, mybir

    f32 = mybir.dt.float32
    bf16 = mybir.dt.bfloat16
    AX = mybir.AxisListType
    OP = mybir.AluOpType
    AF = mybir.ActivationFunctionType

    nc = bacc.Bacc("TRN2", target_bir_lowering=False, debug=False, num_devices=NCORES)

    xT_ext = nc.dram_tensor("xT", [128, NDT, T], bf16, kind="ExternalInput").ap()
    whT_ext = nc.dram_tensor("whT", [128, NDT, WCOLS], bf16, kind="ExternalInput").ap()
    WT_ext = nc.dram_tensor("WT", [128, NDT, DL], bf16, kind="ExternalInput").ap()
    out_ext = nc.dram_tensor("out", [DL, T], bf16, kind="ExternalOutput").ap()

    with tile.TileContext(nc) as tc, ExitStack() as ctx:
        # ---------------- persistent pools ----------------
        pers = ctx.enter_context(tc.tile_pool(name="pers", bufs=1))
        dram = ctx.enter_context(tc.tile_pool(name="dram", bufs=1, space="DRAM"))

        warm_sb = pers.tile([128, 640], bf16)
        xT_sb = pers.tile([128, NDT, T], bf16)
        whT_sb = pers.tile([128, NDT, WCOLS], bf16)
        WT_sb = pers.tile([128, NDT, DL], bf16)
        qT_sb = [[pers.tile([128, 512], bf16, name=f"qT{rb}_{tq}")
                  for tq in range(4)] for rb in range(2)]
        kT_sb = [[pers.tile([128, 512], bf16, name=f"kT{rb}_{tq}")
                  for tq in range(4)] for rb in range(2)]
        # v + 64 ones-columns per (token-tile, head): av lhsT = [v | 1]
        v_sb = pers.tile([128, NTT, HL, 128], bf16)
        aoT_sb = [pers.tile([128, T], bf16, name=f"aoT{rb}") for rb in range(2)]

        qknat = dram.tile([T, 2 * DL], bf16)

        ag_in = [dram.tile([DL, w], bf16, name=f"ag_in{k}")
                 for k, (_, w) in enumerate(BLOCKS)]
        ag_out = [dram.tile([D, w], bf16, name=f"ag_out{k}")
                  for k, (_, w) in enumerate(BLOCKS)]

        # warmup source + the ones columns of v (one strided memset each)
        nc.vector.memset(warm_sb[:], 1.0)
        nc.vector.memset(v_sb[:, :, :, 64:128], 1.0)

        # ---------------- input DMAs (contiguous, split across queues) ----
        # balance the QKV-critical bytes (whT full + x first chunk) across
        # the three DMA-capable queues; all transfers are contiguous per
        # partition thanks to the host pre-tiling
        nc.sync.dma_start(whT_sb[:, 0:4, :], whT_ext[:, 0:4, :])
        nc.scalar.dma_start(whT_sb[:, 4:8, :], whT_ext[:, 4:8, :])
        nc.scalar.dma_start(WT_sb[:], WT_ext)
        for xc in (3, 0, 1, 2):  # match TT_ORDER consumption
            nc.gpsimd.dma_start(
                xT_sb[:, :, 512 * xc : 512 * (xc + 1)],
                xT_ext[:, :, 512 * xc : 512 * (xc + 1)])

        # ---------------- QKV + attention (one pool scope: no pool-close
        # barrier between the phases) ----------------
        with tc.tile_pool(name="scps", bufs=2, space="PSUM") as scps, \
             tc.tile_pool(name="pops", bufs=1, space="PSUM") as pops, \
             tc.tile_pool(name="ypps", bufs=1, space="PSUM") as ypps, \
             tc.tile_pool(name="qsb", bufs=3) as qsb, \
             tc.tile_pool(name="exsb", bufs=4) as exsb, \
             tc.tile_pool(name="posb", bufs=2) as posb, \
             tc.tile_pool(name="aosb", bufs=2) as aosb, \
             tc.tile_pool(name="ysb", bufs=2) as ysb:
            # PE warm-up: dependency-light matmuls while x/whT stream in
            # (uses the po ring so no extra PSUM pool is needed)
            for wu in range(12):
                wt = pops.tile([128, 2 * BANKW], f32, name="po")
                nc.tensor.matmul(wt[:, 0:512], warm_sb[:, 0:128],
                                 warm_sb[:, 128:640], start=True, stop=True)

            done_tq = [0, 0, 0, 0]
            for tt in TT_ORDER:
                ps = scps.tile([128, 2 * BANKW], f32, name="sc")
                for dt_ in range(NDT):
                    lhsT = xT_sb[:, dt_, 128 * tt : 128 * (tt + 1)]
                    nc.tensor.matmul(ps[:, 0:512], lhsT, whT_sb[:, dt_, 0:512],
                                     start=(dt_ == 0), stop=(dt_ == NDT - 1))
                    nc.tensor.matmul(ps[:, 512:768], lhsT, whT_sb[:, dt_, 512:768],
                                     start=(dt_ == 0), stop=(dt_ == NDT - 1))
                # evict q+k raw and v quickly so ps recycles (ACT, 2 instrs)
                qk_sb = qsb.tile([128, 2 * DL], bf16, name="qk_sb")
                nc.scalar.activation(qk_sb[:], ps[:, 0 : 2 * DL], AF.Copy)
                nc.vector.tensor_copy(
                    v_sb[:, tt, :, 0:64],
                    ps[:, 2 * DL : 3 * DL].rearrange("p (h c) -> p h c", c=HD))
                # per-head norms of q and k off the SBUF copy
                sq = qsb.tile([128, 2 * DL], bf16, name="sq")
                nc.vector.tensor_tensor(sq[:], qk_sb[:], qk_sb[:], op=OP.mult)
                ns = qsb.tile([128, 2 * HL], f32, name="ns")
                nc.vector.reduce_sum(
                    ns[:], sq[:].rearrange("p (h c) -> p h c", c=HD), axis=AX.X)
                # sqrt(sumsq/64) = |q|/8 for q; sqrt(sumsq) = |k| for k
                nc.scalar.activation(ns[:, 0:HL], ns[:, 0:HL], AF.Sqrt,
                                     scale=1.0 / 64.0)
                nc.scalar.activation(ns[:, HL : 2 * HL], ns[:, HL : 2 * HL],
                                     AF.Sqrt)
                inv = qsb.tile([128, 2 * HL], f32, name="inv")
                nc.vector.reciprocal(inv[:], ns[:])
                # qst = q * 8/|q| ; kst = k / |k|  (scale folded per head)
                qkst = qsb.tile([128, 2 * DL], bf16, name="qkst")
                nc.vector.tensor_tensor(
                    qkst[:].rearrange("p (h c) -> p h c", c=HD),
                    qk_sb[:].rearrange("p (h c) -> p h c", c=HD),
                    inv[:].rearrange("p (h o) -> p h o", o=1).broadcast_to(
                        (128, 2 * HL, HD)),
                    op=OP.mult)
                nc.sync.dma_start(qknat[128 * tt : 128 * (tt + 1), :], qkst[:])
                done_tq[tt // 4] += 1
                if done_tq[tt // 4] == 4:
                    tq = tt // 4
                    tsl = slice(512 * tq, 512 * (tq + 1))
                    for rb in range(2):
                        nc.sync.dma_start_transpose(
                            qT_sb[rb][tq][:],
                            qknat[tsl, 128 * rb : 128 * (rb + 1)])
                        nc.sync.dma_start_transpose(
                            kT_sb[rb][tq][:],
                            qknat[tsl, 256 + 128 * rb : 256 + 128 * (rb + 1)])

            # ---------- attention + overlapped AG/out-proj ----------
            # av matmuls trail the sc/exp stream by TWO iterations so the
            # in-order PE queue dispatches back-to-back (exp latency hidden).
            # Out-proj runs as one solid 16-matmul burst per chunk.

            def outproj_solid(aoTf, ci):
                """one solid PE run: 16 accumulating matmuls into a borrowed
                scps tile (mh halves in separate banks), evict, DMA out."""
                tok0c, wc = BLOCKS[ci]
                yp = ypps.tile([128, 2 * BANKW], f32, name="yp", tag="yp")
                for mh in range(2):
                    for dt_ in range(NDT):
                        nc.tensor.matmul(
                            yp[:, BANKW * mh : BANKW * mh + wc],
                            WT_sb[:, dt_, 128 * mh : 128 * (mh + 1)],
                            aoTf[:, dt_, 0:wc],
                            start=(dt_ == 0), stop=(dt_ == NDT - 1))
                yst = ysb.tile([128, 2 * BANKW], bf16, name="yst")
                nc.vector.tensor_copy(
                    yst[:, 0 : 2 * wc].rearrange("p (h w) -> p h w", h=2),
                    yp[:].rearrange("p (h w) -> p h w", h=2)[:, :, 0:wc])
                for mh in range(2):
                    nc.sync.dma_start(
                        out_ext[128 * mh : 128 * (mh + 1), tok0c : tok0c + wc],
                        yst[:, wc * mh : wc * (mh + 1)])

            git = 0  # global attention iteration counter
            for bi, (tok0, width) in enumerate(BLOCKS):
                for rb in range(2):
                    po = pops.tile([128, 2 * BANKW], f32, name="po")
                    pend = []  # (ex, j) issued ahead of their av matmuls
                    for idx in range(NTT + 2):
                        if idx < NTT:
                            j = J_ORDER[idx]
                            sc = scps.tile([128, 2 * BANKW], f32, name="sc")
                            jq, jr = j // 4, j % 4
                            qq, qr = tok0 // 512, tok0 % 512
                            for hh in range(2):
                                nc.tensor.matmul(
                                    sc[:, BANKW * hh : BANKW * hh + width],
                                    kT_sb[rb][jq][64 * hh : 64 * (hh + 1),
                                                  128 * jr : 128 * (jr + 1)],
                                    qT_sb[rb][qq][64 * hh : 64 * (hh + 1),
                                                  qr : qr + width],
                                    start=True, stop=True)
                            ex = exsb.tile([128, 2 * BANKW], bf16, name="ex")
                            nc.scalar.activation(
                                ex[:, 0 : 2 * width].rearrange(
                                    "p (h w) -> p h w", h=2),
                                sc[:].rearrange("p (h w) -> p h w",
                                                h=2)[:, :, 0:width],
                                AF.Exp)
                            pend.append(ex)
                        if idx >= 2:
                            jj = J_ORDER[idx - 2]
                            exd = pend.pop(0)
                            for hh in range(2):
                                nc.tensor.matmul(
                                    po[0:128, BANKW * hh : BANKW * hh + width],
                                    v_sb[:, jj, 2 * rb + hh, 0:128],
                                    exd[:, width * hh : width * (hh + 1)],
                                    start=(idx == 2), stop=(idx == NTT + 1))
                            git += 1
                    # evict po (frees the accumulator banks for the next
                    # head-pair), then normalize off SBUF: ao = av/rowsum
                    poc = posb.tile([64, 2 * BANKW], f32, name="poc")
                    nc.vector.tensor_copy(
                        poc[:, 0 : 2 * width].rearrange("p (h w) -> p h w", h=2),
                        po[0:64].rearrange("p (h w) -> p h w", h=2)[:, :, 0:width])
                    rsum = posb.tile([64, 2 * BANKW], f32, name="rsum")
                    nc.vector.tensor_copy(
                        rsum[:, 0 : 2 * width].rearrange("p (h w) -> p h w", h=2),
                        po[64:128].rearrange("p (h w) -> p h w",
                                             h=2)[:, :, 0:width])
                    rinv = posb.tile([64, 2 * BANKW], f32, name="rinv")
                    nc.vector.reciprocal_approx_fast(
                        rinv[:, 0 : 2 * width], rsum[:, 0 : 2 * width])
                    for hh in range(2):
                        nc.vector.tensor_tensor(
                            aoT_sb[rb][64 * hh : 64 * (hh + 1),
                                       tok0 : tok0 + width],
                            poc[:, width * hh : width * (hh + 1)],
                            rinv[:, width * hh : width * (hh + 1)],
                            op=OP.mult)
                # both rb done: ship ao chunk and AllGather it (gpsimd queue
                # orders the DMAs before the collective trigger)
                for rb in range(2):
                    nc.gpsimd.dma_start(
                        ag_in[bi][128 * rb : 128 * (rb + 1), :],
                        aoT_sb[rb][:, tok0 : tok0 + width])
                nc.gpsimd.collective_compute(
                    "AllGather", mybir.AluOpType.bypass,
                    replica_groups=[[0, 1, 2, 3], [4, 5, 6, 7]],
                    ins=[ag_in[bi].opt()], outs=[ag_out[bi].opt()])
                # load the gathered full-feature aoT (sync queue; waits on AG)
                aoTf = aosb.tile([128, NDT, 512], bf16, name="aoTf")
                nc.sync.dma_start(
                    aoTf[:, :, 0:width],
                    ag_out[bi].rearrange("(n p) t -> p n t", p=128))
                if bi < len(WAITS):
                    with tc.tile_wait_until(WAITS[bi]):
                        outproj_solid(aoTf, bi)
                else:
                    outproj_solid(aoTf, bi)

    nc.compile()
    return nc


@functools.lru_cache(maxsize=1)
def _get_nc():
    return _build_bass()


def _mp_normalize_rows(w):
    n = np.linalg.norm(w, axis=-1, keepdims=True)
    n = EPS + n * (1.0 / np.sqrt(w.shape[-1]))
    return (w / n) * (1.0 / np.sqrt(w.shape[-1]))


def _tile_p(a):
    """[128*n, c] row-major -> [128, n, c] with row = n*128 + p."""
    n = a.shape[0] // 128
    return np.ascontiguousarray(a.reshape(n, 128, *a.shape[1:]).swapaxes(0, 1))


def make_in_maps(x, w_qkv, w_out):
    import ml_dtypes

    x = np.asarray(x, dtype=np.float32)
    w_qkv = np.asarray(w_qkv, dtype=np.float32)
    w_out = np.asarray(w_out, dtype=np.float32)

    wq_hat = _mp_normalize_rows(w_qkv)           # (3D, D) row-normalized/32
    wo_hat = _mp_normalize_rows(w_out)           # (D, D)
    woT = np.ascontiguousarray(wo_hat.T)         # (D_in, D_out)

    in_maps = []
    for c in range(NCORES):
        b, g = c // GROUP, c % GROUP
        rows = np.concatenate([
            np.arange(DL * g, DL * (g + 1)),
            D + np.arange(DL * g, DL * (g + 1)),
            2 * D + np.arange(DL * g, DL * (g + 1)),
        ])
        whT = np.ascontiguousarray(wq_hat[rows].T)   # (D, 768)
        in_maps.append({
            "xT": _tile_p(np.ascontiguousarray(x[b].T)).astype(ml_dtypes.bfloat16),
            "whT": _tile_p(whT).astype(ml_dtypes.bfloat16),
            "WT": _tile_p(np.ascontiguousarray(
                woT[:, DL * g : DL * (g + 1)])).astype(ml_dtypes.bfloat16),
        })
    return in_maps


def kernel(x: np.ndarray, w_qkv: np.ndarray, w_out: np.ndarray) -> np.ndarray:
    from concourse.bass_utils import run_bass_kernel_spmd

    in_maps = make_in_maps(x, w_qkv, w_out)
    nc = _get_nc()
    res = run_bass_kernel_spmd(nc, in_maps, core_ids=list(range(NCORES)))

    out = np.empty((B, T, D), dtype=np.float32)
    for c in range(NCORES):
        b, g = c // GROUP, c % GROUP
        out[b][:, DL * g : DL * (g + 1)] = res.results[c]["out"].astype(np.float32).T
    return out


# revision 32
# speedup vs baseline: 1.0419x; 1.0323x over previous
"""Distributed Bass kernel for nn_Attention (B=2, T=2048, D=1024, H=16) on 8 TRN2 cores.

Sharding: core c -> (batch b = c//4, head-group g = c%4, heads 4g..4g+3).
QKV tensor-parallel over heads; out-proj COLUMN-parallel: per token chunk,
AllGather the (small, 128-256KB) normalized attention outputs across the
4-core group, then each core computes its own 256-col slice of the final
output locally -- no ReduceScatter, no collective after the last out-proj,
and ~4x less collective traffic than the row-parallel+RS formulation.

Key mechanics (see git history of this session for the journey):
  - rowsum rides the av matmul as 64 ones-columns in the lhsT (m: 65->128;
    matmul time is free-dim-bound so the row sums are free), so softmax
    normalize is two aligned PSUM->SBUF copies + a fast ~18-bit DVE
    reciprocal + multiply, entirely off the PE/ACT critical engines.
  - av matmuls trail the sc/exp stream by TWO iterations so the in-order PE
    queue never head-of-line blocks on the exp (ACT) latency.
  - scalar engine runs only Exp in attention (exactly one act-table load at
    the phase transition); q/k eviction+norms use ACT-copy/sqrt + DVE.
  - one PSUM pool scope across QKV and attention (same tile names): no
    all-engine pool-close barrier between the phases.
  - block order [1536x256, 0..1536 as 3x512, 1792x256]: the expensive cold
    first AllGather is triggered ~20us earlier on a cheap block and the
    final chunk's AllGather is the small one; QKV processes token tiles in
    TT_ORDER (last quarter first) so the q/k transposes needed by the first
    attention block are ready at the transition, and the attention visits
    key tiles in J_ORDER so the late-transposed quarter is needed last.
  - out-proj runs as one solid 16-matmul burst per chunk, placed via
    tc.tile_wait_until scheduler hints (the cost model underestimates the
    0.4-11us trigger delay + 9-36us execution of the AllGathers here, so
    dependency-driven placement would stall the in-order PE queue).
  - host pre-tiles xT/whT/WT to [128, n, ...] so all input DMAs are
    contiguous per partition; xT chunks are DMAed in TT_ORDER consumption
    order; whT is split across two DMA queues.
"""

import functools
import numpy as np
from contextlib import ExitStack

B, T, D, H, HD = 2, 2048, 1024, 16, 64
EPS = 1e-4
NCORES, GROUP = 8, 4
HL = H // GROUP          # heads per core = 4
DL = HL * HD             # local feature cols = 256
NTT = T // 128           # 16 token tiles
NDT = D // 128           # 8 d tiles
WCOLS = 3 * DL           # 768 qkv output cols per core

# attention q-blocks (tok0, width); the two row-tiled score matmuls write
# PSUM offsets {0, BANKW} so each lands in its own bank even at width 256.
BLOCKS = [(1536, 256), (0, 512), (512, 512), (1024, 512), (1792, 256)]
BANKW = 512
# QKV token-tile processing order: the 1536-2047 quarter first so its q/k
# transposes are ready when attention (which starts with those queries)
# begins; the 1024-1535 quarter last (only needed late).
TT_ORDER = [12, 13, 14, 15] + list(range(8)) + [8, 9, 10, 11]
# attention key-tile visit order: defer the 1024-1535 keys (their transposes
# land right at the QKV->attention boundary)
J_ORDER = list(range(8)) + [12, 13, 14, 15] + [8, 9, 10, 11]
# scheduler not-before hints (in ms of simulated time) for each chunk's
# out-proj burst: the cost model underestimates the (cold) AllGather latency,
# so without these the scheduler places the bursts too early and the in-order
# PE queue stalls on the aoTf load.
WAITS = [0.132, 0.158, 0.188, 0.235]


def _build_bass():
    import concourse.bass as bass
    import concourse.tile as tile
    from concourse import bacc, mybir

    f32 = mybir.dt.float32
    bf16 = mybir.dt.bfloat16
    AX = mybir.AxisListType
    OP = mybir.AluOpType
    AF = mybir.ActivationFunctionType

    nc = bacc.Bacc("TRN2", target_bir_lowering=False, debug=False, num_devices=NCORES)

    xT_ext = nc.dram_tensor("xT", [128, NDT, T], bf16, kind="ExternalInput").ap()
    whT_ext = nc.dram_tensor("whT", [128, NDT, WCOLS], bf16, kind="ExternalInput").ap()
    WT_ext = nc.dram_tensor("WT", [128, NDT, DL], bf16, kind="ExternalInput").ap()
    out_ext = nc.dram_tensor("out", [DL, T], bf16, kind="ExternalOutput").ap()

    with tile.TileContext(nc) as tc, ExitStack() as ctx:
        # ---------------- persistent pools ----------------
        pers = ctx.enter_context(tc.tile_pool(name="pers", bufs=1))
        dram = ctx.enter_context(tc.tile_pool(name="dram", bufs=1, space="DRAM"))

        warm_sb = pers.tile([128, 640], bf16)
        xT_sb = pers.tile([128, NDT, T], bf16)
        whT_sb = pers.tile([128, NDT, WCOLS], bf16)
        WT_sb = pers.tile([128, NDT, DL], bf16)
        qT_sb = [[pers.tile([128, 512], bf16, name=f"qT{rb}_{tq}")
                  for tq in range(4)] for rb in range(2)]
        kT_sb = [[pers.tile([128, 512], bf16, name=f"kT{rb}_{tq}")
                  for tq in range(4)] for rb in range(2)]
        # v + 64 ones-columns per (token-tile, head): av lhsT = [v | 1]
        v_sb = pers.tile([128, NTT, HL, 128], bf16)
        aoT_sb = [pers.tile([128, T], bf16, name=f"aoT{rb}") for rb in range(2)]

        qknat = dram.tile([T, 2 * DL], bf16)

        ag_in = [dram.tile([DL, w], bf16, name=f"ag_in{k}")
                 for k, (_, w) in enumerate(BLOCKS)]
        ag_out = [dram.tile([D, w], bf16, name=f"ag_out{k}")
                  for k, (_, w) in enumerate(BLOCKS)]

        # warmup source + the ones columns of v (one strided memset each)
        nc.vector.memset(warm_sb[:], 1.0)
        nc.vector.memset(v_sb[:, :, :, 64:128], 1.0)

        # ---------------- input DMAs (contiguous, split across queues) ----
        # balance the QKV-critical bytes (whT full + x first chunk) across
        # the three DMA-capable queues; all transfers are contiguous per
        # partition thanks to the host pre-tiling
        nc.sync.dma_start(whT_sb[:, 0:4, :], whT_ext[:, 0:4, :])
        nc.scalar.dma_start(whT_sb[:, 4:8, :], whT_ext[:, 4:8, :])
        nc.scalar.dma_start(WT_sb[:], WT_ext)
        for xc in (3, 0, 1, 2):  # match TT_ORDER consumption
            nc.gpsimd.dma_start(
                xT_sb[:, :, 512 * xc : 512 * (xc + 1)],
                xT_ext[:, :, 512 * xc : 512 * (xc + 1)])

        # ---------------- QKV + attention (one pool scope: no pool-close
        # barrier between the phases) ----------------
        with tc.tile_pool(name="scps", bufs=2, space="PSUM") as scps, \
             tc.tile_pool(name="pops", bufs=1, space="PSUM") as pops, \
             tc.tile_pool(name="ypps", bufs=1, space="PSUM") as ypps, \
             tc.tile_pool(name="qsb", bufs=3) as qsb, \
             tc.tile_pool(name="exsb", bufs=4) as exsb, \
             tc.tile_pool(name="posb", bufs=2) as posb, \
             tc.tile_pool(name="aosb", bufs=2) as aosb, \
             tc.tile_pool(name="ysb", bufs=2) as ysb:
            # PE warm-up: dependency-light matmuls while x/whT stream in
            # (uses the po ring so no extra PSUM pool is needed)
            for wu in range(12):
                wt = pops.tile([128, 2 * BANKW], f32, name="po")
                nc.tensor.matmul(wt[:, 0:512], warm_sb[:, 0:128],
                                 warm_sb[:, 128:640], start=True, stop=True)

            done_tq = [0, 0, 0, 0]
            for tt in TT_ORDER:
                ps = scps.tile([128, 2 * BANKW], f32, name="sc")
                for dt_ in range(NDT):
                    lhsT = xT_sb[:, dt_, 128 * tt : 128 * (tt + 1)]
                    nc.tensor.matmul(ps[:, 0:512], lhsT, whT_sb[:, dt_, 0:512],
                                     start=(dt_ == 0), stop=(dt_ == NDT - 1))
                    nc.tensor.matmul(ps[:, 512:768], lhsT, whT_sb[:, dt_, 512:768],
                                     start=(dt_ == 0), stop=(dt_ == NDT - 1))
                # evict q+k raw and v quickly so ps recycles (ACT, 2 instrs)
                qk_sb = qsb.tile([128, 2 * DL], bf16, name="qk_sb")
                nc.scalar.activation(qk_sb[:], ps[:, 0 : 2 * DL], AF.Copy)
                nc.vector.tensor_copy(
                    v_sb[:, tt, :, 0:64],
                    ps[:, 2 * DL : 3 * DL].rearrange("p (h c) -> p h c", c=HD))
                # per-head norms of q and k off the SBUF copy
                sq = qsb.tile([128, 2 * DL], bf16, name="sq")
                nc.vector.tensor_tensor(sq[:], qk_sb[:], qk_sb[:], op=OP.mult)
                ns = qsb.tile([128, 2 * HL], f32, name="ns")
                nc.vector.reduce_sum(
                    ns[:], sq[:].rearrange("p (h c) -> p h c", c=HD), axis=AX.X)
                # sqrt(sumsq/64) = |q|/8 for q; sqrt(sumsq) = |k| for k
                nc.scalar.activation(ns[:, 0:HL], ns[:, 0:HL], AF.Sqrt,
                                     scale=1.0 / 64.0)
                nc.scalar.activation(ns[:, HL : 2 * HL], ns[:, HL : 2 * HL],
                                     AF.Sqrt)
                inv = qsb.tile([128, 2 * HL], f32, name="inv")
                nc.vector.reciprocal(inv[:], ns[:])
                # qst = q * 8/|q| ; kst = k / |k|  (scale folded per head)
                qkst = qsb.tile([128, 2 * DL], bf16, name="qkst")
                nc.vector.tensor_tensor(
                    qkst[:].rearrange("p (h c) -> p h c", c=HD),
                    qk_sb[:].rearrange("p (h c) -> p h c", c=HD),
                    inv[:].rearrange("p (h o) -> p h o", o=1).broadcast_to(
                        (128, 2 * HL, HD)),
                    op=OP.mult)
                nc.sync.dma_start(qknat[128 * tt : 128 * (tt + 1), :], qkst[:])
                done_tq[tt // 4] += 1
                if done_tq[tt // 4] == 4:
                    tq = tt // 4
                    tsl = slice(512 * tq, 512 * (tq + 1))
                    for rb in range(2):
                        nc.sync.dma_start_transpose(
                            qT_sb[rb][tq][:],
                            qknat[tsl, 128 * rb : 128 * (rb + 1)])
                        nc.sync.dma_start_transpose(
                            kT_sb[rb][tq][:],
                            qknat[tsl, 256 + 128 * rb : 256 + 128 * (rb + 1)])

            # ---------- attention + overlapped AG/out-proj ----------
            # av matmuls trail the sc/exp stream by TWO iterations so the
            # in-order PE queue dispatches back-to-back (exp latency hidden).
            # Out-proj runs as one solid 16-matmul burst per chunk.

            def outproj_solid(aoTf, ci):
                """one solid PE run: 16 accumulating matmuls into a borrowed
                scps tile (mh halves in separate banks), evict, DMA out."""
                tok0c, wc = BLOCKS[ci]
                yp = ypps.tile([128, 2 * BANKW], f32, name="yp", tag="yp")
                for mh in range(2):
                    for dt_ in range(NDT):
                        nc.tensor.matmul(
                            yp[:, BANKW * mh : BANKW * mh + wc],
                            WT_sb[:, dt_, 128 * mh : 128 * (mh + 1)],
                            aoTf[:, dt_, 0:wc],
                            start=(dt_ == 0), stop=(dt_ == NDT - 1))
                yst = ysb.tile([128, 2 * BANKW], bf16, name="yst")
                nc.vector.tensor_copy(
                    yst[:, 0 : 2 * wc].rearrange("p (h w) -> p h w", h=2),
                    yp[:].rearrange("p (h w) -> p h w", h=2)[:, :, 0:wc])
                for mh in range(2):
                    nc.sync.dma_start(
                        out_ext[128 * mh : 128 * (mh + 1), tok0c : tok0c + wc],
                        yst[:, wc * mh : wc * (mh + 1)])

            git = 0  # global attention iteration counter
            for bi, (tok0, width) in enumerate(BLOCKS):
                for rb in range(2):
                    po = pops.tile([128, 2 * BANKW], f32, name="po")
                    pend = []  # (ex, j) issued ahead of their av matmuls
                    for idx in range(NTT + 2):
                        if idx < NTT:
                            j = J_ORDER[idx]
                            sc = scps.tile([128, 2 * BANKW], f32, name="sc")
                            jq, jr = j // 4, j % 4
                            qq, qr = tok0 // 512, tok0 % 512
                            for hh in range(2):
                                nc.tensor.matmul(
                                    sc[:, BANKW * hh : BANKW * hh + width],
                                    kT_sb[rb][jq][64 * hh : 64 * (hh + 1),
                                                  128 * jr : 128 * (jr + 1)],
                                    qT_sb[rb][qq][64 * hh : 64 * (hh + 1),
                                                  qr : qr + width],
                                    start=True, stop=True)
                            ex = exsb.tile([128, 2 * BANKW], bf16, name="ex")
                            nc.scalar.activation(
                                ex[:, 0 : 2 * width].rearrange(
                                    "p (h w) -> p h w", h=2),
                                sc[:].rearrange("p (h w) -> p h w",
                                                h=2)[:, :, 0:width],
                                AF.Exp)
                            pend.append(ex)
                        if idx >= 2:
                            jj = J_ORDER[idx - 2]
                            exd = pend.pop(0)
                            for hh in range(2):
                                nc.tensor.matmul(
                                    po[0:128, BANKW * hh : BANKW * hh + width],
                                    v_sb[:, jj, 2 * rb + hh, 0:128],
                                    exd[:, width * hh : width * (hh + 1)],
                                    start=(idx == 2), stop=(idx == NTT + 1))
                            git += 1
                    # evict po (frees the accumulator banks for the next
                    # head-pair), then normalize off SBUF: ao = av/rowsum
                    poc = posb.tile([64, 2 * BANKW], f32, name="poc")
                    nc.vector.tensor_copy(
                        poc[:, 0 : 2 * width].rearrange("p (h w) -> p h w", h=2),
                        po[0:64].rearrange("p (h w) -> p h w", h=2)[:, :, 0:width])
                    rsum = posb.tile([64, 2 * BANKW], f32, name="rsum")
                    nc.vector.tensor_copy(
                        rsum[:, 0 : 2 * width].rearrange("p (h w) -> p h w", h=2),
                        po[64:128].rearrange("p (h w) -> p h w",
                                             h=2)[:, :, 0:width])
                    rinv = posb.tile([64, 2 * BANKW], f32, name="rinv")
                    nc.vector.reciprocal_approx_fast(
                        rinv[:, 0 : 2 * width], rsum[:, 0 : 2 * width])
                    for hh in range(2):
                        nc.vector.tensor_tensor(
                            aoT_sb[rb][64 * hh : 64 * (hh + 1),
                                       tok0 : tok0 + width],
                            poc[:, width * hh : width * (hh + 1)],
                            rinv[:, width * hh : width * (hh + 1)],
                            op=OP.mult)
                    # ship this head-pair's ao rows immediately so the AG
                    # trigger only waits on the second rb's short chain
                    nc.gpsimd.dma_start(
                        ag_in[bi][128 * rb : 128 * (rb + 1), :],
                        aoT_sb[rb][:, tok0 : tok0 + width])
                nc.gpsimd.collective_compute(
                    "AllGather", mybir.AluOpType.bypass,
                    replica_groups=[[0, 1, 2, 3], [4, 5, 6, 7]],
                    ins=[ag_in[bi].opt()], outs=[ag_out[bi].opt()])
                # load the gathered full-feature aoT (sync queue; waits on AG)
                aoTf = aosb.tile([128, NDT, 512], bf16, name="aoTf")
                nc.sync.dma_start(
                    aoTf[:, :, 0:width],
                    ag_out[bi].rearrange("(n p) t -> p n t", p=128))
                if bi < len(WAITS):
                    with tc.tile_wait_until(WAITS[bi]):
                        outproj_solid(aoTf, bi)
                else:
                    outproj_solid(aoTf, bi)

    nc.compile()
    return nc


@functools.lru_cache(maxsize=1)
def _get_nc():
    return _build_bass()


def _mp_normalize_rows(w):
    n = np.linalg.norm(w, axis=-1, keepdims=True)
    n = EPS + n * (1.0 / np.sqrt(w.shape[-1]))
    return (w / n) * (1.0 / np.sqrt(w.shape[-1]))


def _tile_p(a):
    """[128*n, c] row-major -> [128, n, c] with row = n*128 + p."""
    n = a.shape[0] // 128
    return np.ascontiguousarray(a.reshape(n, 128, *a.shape[1:]).swapaxes(0, 1))


def make_in_maps(x, w_qkv, w_out):
    import ml_dtypes

    x = np.asarray(x, dtype=np.float32)
    w_qkv = np.asarray(w_qkv, dtype=np.float32)
    w_out = np.asarray(w_out, dtype=np.float32)

    wq_hat = _mp_normalize_rows(w_qkv)           # (3D, D) row-normalized/32
    wo_hat = _mp_normalize_rows(w_out)           # (D, D)
    woT = np.ascontiguousarray(wo_hat.T)         # (D_in, D_out)

    in_maps = []
    for c in range(NCORES):
        b, g = c // GROUP, c % GROUP
        rows = np.concatenate([
            np.arange(DL * g, DL * (g + 1)),
            D + np.arange(DL * g, DL * (g + 1)),
            2 * D + np.arange(DL * g, DL * (g + 1)),
        ])
        whT = np.ascontiguousarray(wq_hat[rows].T)   # (D, 768)
        in_maps.append({
            "xT": _tile_p(np.ascontiguousarray(x[b].T)).astype(ml_dtypes.bfloat16),
            "whT": _tile_p(whT).astype(ml_dtypes.bfloat16),
            "WT": _tile_p(np.ascontiguousarray(
                woT[:, DL * g : DL * (g + 1)])).astype(ml_dtypes.bfloat16),
        })
    return in_maps


def kernel(x: np.ndarray, w_qkv: np.ndarray, w_out: np.ndarray) -> np.ndarray:
    from concourse.bass_utils import run_bass_kernel_spmd

    in_maps = make_in_maps(x, w_qkv, w_out)
    nc = _get_nc()
    res = run_bass_kernel_spmd(nc, in_maps, core_ids=list(range(NCORES)))

    out = np.empty((B, T, D), dtype=np.float32)
    for c in range(NCORES):
        b, g = c // GROUP, c % GROUP
        out[b][:, DL * g : DL * (g + 1)] = res.results[c]["out"].astype(np.float32).T
    return out
